# revision 3
# baseline (speedup 1.0000x reference)
"""EvidentialGNN3D Trainium2 kernel (8 NeuronCores, SPMD).

Strategy: edges sharded by destination-node range (25000 nodes/core). On the
host (index marshalling only — no float math): each core's edges are arranged
into a ragged "layer" structure — nodes sorted by in-degree, edge j of node
rank r lands at slot (j, r) — so the segment-sum over incoming edges becomes
a short sequence of dense contiguous vector adds on device. All floating
point compute (edge geometry, W_msg projections, tanh messages, segment
sums, state updates, graph reduction, output net, softplus) runs on the
NeuronCores across 4 SPMD launches; the host only permutes/replicates arrays
between launches (gather of rows by precomputed integer indices).
"""
import os
import sys

for _p in ("/opt/trn_rl_repo", "/root/.axon_site"):
    if _p not in sys.path:
        sys.path.insert(0, _p)

import numpy as np

import concourse.bacc as bacc
import concourse.mybir as mybir
import concourse.tile as tile
from concourse.bass_utils import run_bass_kernel_spmd

N = 200_000
E = 6_400_000
G = 2_000
SD = 10
NC = 8
SLICE = N // NC            # 25000 nodes per core
SLABC = 196                # slab cols: 196*128 = 25088 >= 25000
SLAB = SLABC * 128
CH = 512                   # cols per processing chunk
F32 = mybir.dt.float32

LAST_EXEC_NS = 0
_PROG_CACHE = {}


def _trace_on():
    return bool(os.environ.get("GNN_TRACE"))


def _maybe_install_trace_hook():
    if not _trace_on():
        return
    import types
    import antenv
    if "antenv.axon_hooks" in sys.modules:
        return
    mod = types.ModuleType("antenv.axon_hooks")
    _h = [None]
    mod.set_axon_ntff_profile_hook = lambda h: _h.__setitem__(0, h)
    mod.get_axon_ntff_profile_hook = lambda: _h[0]
    sys.modules["antenv.axon_hooks"] = mod
    antenv.axon_hooks = mod
    from trn_agent_boot.trn_boot import _ntff_profile_via_ctypes
    mod.set_axon_ntff_profile_hook(
        _ntff_profile_via_ctypes("/opt/axon/libaxon_pjrt.so"))


def _run(nc, in_maps, core_ids):
    global LAST_EXEC_NS
    res = run_bass_kernel_spmd(nc, in_maps, core_ids, trace=_trace_on())
    if res.exec_time_ns:
        LAST_EXEC_NS += int(res.exec_time_ns)
    return res.results


def _chunks(total, step):
    return [(a, min(a + step, total)) for a in range(0, total, step)]


# ---------------------------------------------------------------- programs

def build_geo(tc_cols):
    """in: len[128,TC], cf[128,TC,3], ct[128,TC,3], ev[128,TC,3],
    wg[128,40] (wg[:,k*10+c] = W_msg[10+k, c] replicated)
    out: gp[128,TC,10] = len*W0 + sum|cf|*W1 + (cf.ct)*W2 + (cf.ev)*W3
    """
    key = ("geo", tc_cols)
    if key in _PROG_CACHE:
        return _PROG_CACHE[key]
    nc = bacc.Bacc("TRN2", target_bir_lowering=False, debug=False)
    t_len = nc.dram_tensor("len", [128, tc_cols], F32, kind="ExternalInput")
    t_cf = nc.dram_tensor("cf", [128, tc_cols, 3], F32, kind="ExternalInput")
    t_ct = nc.dram_tensor("ct", [128, tc_cols, 3], F32, kind="ExternalInput")
    t_ev = nc.dram_tensor("ev", [128, tc_cols, 3], F32, kind="ExternalInput")
    t_wg = nc.dram_tensor("wg", [128, 40], F32, kind="ExternalInput")
    t_gp = nc.dram_tensor("gp", [128, tc_cols, 10], F32, kind="ExternalOutput")
    AL = mybir.AluOpType
    with tile.TileContext(nc) as tc:
        with (
            tc.tile_pool(name="io", bufs=2) as iop,
            tc.tile_pool(name="w", bufs=1) as wp,
            tc.tile_pool(name="sc", bufs=2) as scp,
        ):
            w_t = wp.tile([128, 40], F32)
            nc.sync.dma_start(out=w_t[:], in_=t_wg[:])
            for a, b in _chunks(tc_cols, CH):
                c = b - a
                len_t = iop.tile([128, CH], F32, tag="len")
                cf_t = iop.tile([128, CH, 3], F32, tag="cf")
                ct_t = iop.tile([128, CH, 3], F32, tag="ct")
                ev_t = iop.tile([128, CH, 3], F32, tag="ev")
                nc.sync.dma_start(out=len_t[:, :c], in_=t_len[:, a:b])
                nc.sync.dma_start(out=cf_t[:, :c], in_=t_cf[:, a:b])
                nc.sync.dma_start(out=ct_t[:, :c], in_=t_ct[:, a:b])
                nc.sync.dma_start(out=ev_t[:, :c], in_=t_ev[:, a:b])
                # components
                prod = scp.tile([128, CH, 3], F32, tag="prod")
                d1 = scp.tile([128, CH], F32, tag="d1")
                d2 = scp.tile([128, CH], F32, tag="d2")
                ab = scp.tile([128, CH, 3], F32, tag="ab")
                asum = scp.tile([128, CH], F32, tag="asum")
                nc.vector.tensor_tensor(out=prod[:, :c], in0=cf_t[:, :c],
                                        in1=ct_t[:, :c], op=AL.mult)
                nc.vector.tensor_reduce(out=d1[:, :c], in_=prod[:, :c],
                                        axis=mybir.AxisListType.X, op=AL.add)
                nc.vector.tensor_tensor(out=prod[:, :c], in0=cf_t[:, :c],
                                        in1=ev_t[:, :c], op=AL.mult)
                nc.vector.tensor_reduce(out=d2[:, :c], in_=prod[:, :c],
                                        axis=mybir.AxisListType.X, op=AL.add)
                nc.vector.scalar_tensor_tensor(out=ab[:, :c], in0=cf_t[:, :c],
                                               scalar=-1.0, in1=cf_t[:, :c],
                                               op0=AL.mult, op1=AL.max)
                nc.vector.tensor_reduce(out=asum[:, :c], in_=ab[:, :c],
                                        axis=mybir.AxisListType.X, op=AL.add)
                gp_t = iop.tile([128, CH, 10], F32, tag="gp")
                s0 = scp.tile([128, CH], F32, tag="s0")
                s1 = scp.tile([128, CH], F32, tag="s1")
                comps = [len_t, asum, d1, d2]
                for cc in range(10):
                    nc.vector.tensor_scalar_mul(
                        out=s0[:, :c], in0=comps[0][:, :c],
                        scalar1=w_t[:, cc:cc + 1])
                    nc.vector.scalar_tensor_tensor(
                        out=s1[:, :c], in0=comps[1][:, :c],
                        scalar=w_t[:, 10 + cc:11 + cc], in1=s0[:, :c],
                        op0=AL.mult, op1=AL.add)
                    nc.vector.scalar_tensor_tensor(
                        out=s0[:, :c], in0=comps[2][:, :c],
                        scalar=w_t[:, 20 + cc:21 + cc], in1=s1[:, :c],
                        op0=AL.mult, op1=AL.add)
                    nc.vector.scalar_tensor_tensor(
                        out=gp_t[:, :c, cc], in0=comps[3][:, :c],
                        scalar=w_t[:, 30 + cc:31 + cc], in1=s0[:, :c],
                        op0=AL.mult, op1=AL.add)
                nc.sync.dma_start(out=t_gp[:, a:b], in_=gp_t[:, :c])
    nc.compile()
    _PROG_CACHE[key] = nc
    return nc


def build_round(tc_cols, layers):
    """One message-passing round.
    in: gp[128,TC,10], gath[128,TC,10], state_in[128,196,10],
        ws[128,110] (ws[:,d*10+c] = W_msg[d,c]; ws[:,100+c] = b_msg[c])
    out: state[128,196,10] = state_in + acc, proj[128,196,10] = state@Ws + b
    layers: list of (base_col, ncols) per layer j (acc cols 0..ncols-1).
    """
    key = ("round", tc_cols, tuple(layers))
    if key in _PROG_CACHE:
        return _PROG_CACHE[key]
    nc = bacc.Bacc("TRN2", target_bir_lowering=False, debug=False)
    t_gp = nc.dram_tensor("gp", [128, tc_cols, 10], F32, kind="ExternalInput")
    t_ga = nc.dram_tensor("gath", [128, tc_cols, 10], F32, kind="ExternalInput")
    t_si = nc.dram_tensor("state_in", [128, SLABC, 10], F32, kind="ExternalInput")
    t_ws = nc.dram_tensor("ws", [128, 110], F32, kind="ExternalInput")
    t_so = nc.dram_tensor("state", [128, SLABC, 10], F32, kind="ExternalOutput")
    t_pr = nc.dram_tensor("proj", [128, SLABC, 10], F32, kind="ExternalOutput")
    AL = mybir.AluOpType
    # map: chunk -> list of (msg_col_rel, acc_col, ncols)
    seg_by_chunk = []
    for a, b in _chunks(tc_cols, CH):
        segs = []
        for (base, ncols) in layers:
            lo = max(a, base)
            hi = min(b, base + ncols)
            if lo < hi:
                segs.append((lo - a, lo - base, hi - lo))
        seg_by_chunk.append(((a, b), segs))
    with tile.TileContext(nc) as tc:
        with (
            tc.tile_pool(name="io", bufs=2) as iop,
            tc.tile_pool(name="acc", bufs=1) as accp,
            tc.tile_pool(name="w", bufs=1) as wp,
            tc.tile_pool(name="sc", bufs=2) as scp,
        ):
            w_t = wp.tile([128, 110], F32)
            nc.sync.dma_start(out=w_t[:], in_=t_ws[:])
            acc = accp.tile([128, SLABC, 10], F32)
            nc.vector.memset(acc[:], 0.0)
            for (a, b), segs in seg_by_chunk:
                c = b - a
                gp_t = iop.tile([128, CH, 10], F32, tag="gp")
                ga_t = iop.tile([128, CH, 10], F32, tag="ga")
                nc.sync.dma_start(out=gp_t[:, :c], in_=t_gp[:, a:b])
                nc.sync.dma_start(out=ga_t[:, :c], in_=t_ga[:, a:b])
                nc.vector.tensor_tensor(out=gp_t[:, :c], in0=gp_t[:, :c],
                                        in1=ga_t[:, :c], op=AL.add)
                msg = iop.tile([128, CH, 10], F32, tag="msg")
                nc.scalar.activation(out=msg[:, :c], in_=gp_t[:, :c],
                                     func=mybir.ActivationFunctionType.Tanh)
                for (mrel, acol, ncols) in segs:
                    nc.vector.tensor_tensor(
                        out=acc[:, acol:acol + ncols],
                        in0=acc[:, acol:acol + ncols],
                        in1=msg[:, mrel:mrel + ncols], op=AL.add)
            st = accp.tile([128, SLABC, 10], F32, tag="st")
            si_t = accp.tile([128, SLABC, 10], F32, tag="si")
            nc.sync.dma_start(out=si_t[:], in_=t_si[:])
            nc.vector.tensor_tensor(out=st[:], in0=si_t[:], in1=acc[:],
                                    op=AL.add)
            nc.sync.dma_start(out=t_so[:], in_=st[:])
            # proj = state @ W_s + b  (DVE FMA per output channel)
            pr = accp.tile([128, SLABC, 10], F32, tag="pr")
            s0 = scp.tile([128, SLABC], F32, tag="p0")
            s1 = scp.tile([128, SLABC], F32, tag="p1")
            for cc in range(10):
                nc.vector.tensor_scalar_mul(out=s0[:], in0=st[:, :, 0],
                                            scalar1=w_t[:, cc:cc + 1])
                cur, nxt = s0, s1
                for d in range(1, 10):
                    nc.vector.scalar_tensor_tensor(
                        out=nxt[:], in0=st[:, :, d],
                        scalar=w_t[:, d * 10 + cc:d * 10 + cc + 1],
                        in1=cur[:], op0=AL.mult, op1=AL.add)
                    cur, nxt = nxt, cur
                nc.vector.tensor_scalar_add(out=pr[:, :, cc], in0=cur[:],
                                            scalar1=w_t[:, 100 + cc:101 + cc])
            nc.sync.dma_start(out=t_pr[:], in_=pr[:])
    nc.compile()
    _PROG_CACHE[key] = nc
    return nc


def build_final(tg_cols, glayers):
    """Graph reduction + output net + softplus transforms (single core).
    in: rows[128,TG,10] (host-marshalled state rows in graph-layer order),
        wo[128,44] (wo[:,d*4+c] = W_out[d,c]; wo[:,40+c] = b_out[c])
    out: evout[128,16,4]  (graph g at (g%128, g//128))
    """
    key = ("final", tg_cols, tuple(glayers))
    if key in _PROG_CACHE:
        return _PROG_CACHE[key]
    nc = bacc.Bacc("TRN2", target_bir_lowering=False, debug=False)
    t_rows = nc.dram_tensor("rows", [128, tg_cols, 10], F32, kind="ExternalInput")
    t_wo = nc.dram_tensor("wo", [128, 44], F32, kind="ExternalInput")
    t_out = nc.dram_tensor("evout", [128, 16, 4], F32, kind="ExternalOutput")
    AL = mybir.AluOpType
    AF = mybir.ActivationFunctionType
    GC = 16  # 2048 graph slots
    seg_by_chunk = []
    for a, b in _chunks(tg_cols, CH):
        segs = []
        for (base, ncols) in glayers:
            lo = max(a, base)
            hi = min(b, base + ncols)
            if lo < hi:
                segs.append((lo - a, lo - base, hi - lo))
        seg_by_chunk.append(((a, b), segs))
    with tile.TileContext(nc) as tc:
        with (
            tc.tile_pool(name="io", bufs=3) as iop,
            tc.tile_pool(name="acc", bufs=1) as accp,
            tc.tile_pool(name="sc", bufs=2) as scp,
        ):
            w_t = accp.tile([128, 44], F32, tag="w")
            nc.sync.dma_start(out=w_t[:], in_=t_wo[:])
            gs = accp.tile([128, GC, 10], F32)
            nc.vector.memset(gs[:], 0.0)
            for (a, b), segs in seg_by_chunk:
                c = b - a
                r_t = iop.tile([128, CH, 10], F32, tag="r")
                nc.sync.dma_start(out=r_t[:, :c], in_=t_rows[:, a:b])
                for (mrel, acol, ncols) in segs:
                    nc.vector.tensor_tensor(
                        out=gs[:, acol:acol + ncols],
                        in0=gs[:, acol:acol + ncols],
                        in1=r_t[:, mrel:mrel + ncols], op=AL.add)
            # ev = gs @ W_out + b_out
            ev = accp.tile([128, GC, 4], F32, tag="ev")
            s0 = scp.tile([128, GC], F32, tag="f0")
            s1 = scp.tile([128, GC], F32, tag="f1")
            for cc in range(4):
                nc.vector.tensor_scalar_mul(out=s0[:], in0=gs[:, :, 0],
                                            scalar1=w_t[:, cc:cc + 1])
                cur, nxt = s0, s1
                for d in range(1, 10):
                    nc.vector.scalar_tensor_tensor(
                        out=nxt[:], in0=gs[:, :, d],
                        scalar=w_t[:, d * 4 + cc:d * 4 + cc + 1],
                        in1=cur[:], op0=AL.mult, op1=AL.add)
                    cur, nxt = nxt, cur
                nc.vector.tensor_scalar_add(out=ev[:, :, cc], in0=cur[:],
                                            scalar1=w_t[:, 40 + cc:41 + cc])
            # outputs: gamma, sp(v), sp(alpha)+1, sp(beta)
            out_t = accp.tile([128, GC, 4], F32, tag="out")
            nc.vector.tensor_copy(out=out_t[:, :, 0], in_=ev[:, :, 0])
            # softplus(x) = relu(x) + log1p(exp(-|x|))
            sp = scp.tile([128, GC], F32, tag="sp")
            ex = scp.tile([128, GC], F32, tag="ex")
            rl = scp.tile([128, GC], F32, tag="rl")
            for cc in (1, 2, 3):
                nc.vector.scalar_tensor_tensor(
                    out=sp[:], in0=ev[:, :, cc], scalar=-1.0,
                    in1=ev[:, :, cc], op0=AL.mult, op1=AL.min)  # -|x|
                nc.scalar.activation(out=ex[:], in_=sp[:], func=AF.Exp)
                nc.scalar.activation(out=sp[:], in_=ex[:], func=AF.Ln,
                                     bias=1.0)                  # log(e+1)
                nc.vector.tensor_scalar_max(out=rl[:], in0=ev[:, :, cc],
                                            scalar1=0.0)        # relu(x)
                if cc == 2:
                    nc.vector.tensor_tensor(out=rl[:], in0=rl[:], in1=sp[:],
                                            op=AL.add)
                    nc.vector.tensor_scalar_add(out=out_t[:, :, cc],
                                                in0=rl[:], scalar1=1.0)
                else:
                    nc.vector.tensor_tensor(out=out_t[:, :, cc], in0=rl[:],
                                            in1=sp[:], op=AL.add)
            nc.sync.dma_start(out=t_out[:], in_=out_t[:])
    nc.compile()
    _PROG_CACHE[key] = nc
    return nc


# ---------------------------------------------------------------- host prep

def _prep(node_coordinates, edge_lengths, edge_vectors, node_from, node_to,
          node_graph_index):
    """Index-only marshalling: build per-core layer-slot arrays."""
    nf = np.asarray(node_from).astype(np.int64).ravel()
    nt = np.asarray(node_to).astype(np.int64).ravel()
    ngi = np.asarray(node_graph_index).astype(np.int64).ravel()
    coords = np.asarray(node_coordinates, dtype=np.float32)
    elen = np.asarray(edge_lengths, dtype=np.float32).ravel()
    evec = np.asarray(edge_vectors, dtype=np.float32)

    core_of = nt // SLICE
    per_core = []
    all_cj = []
    for k in range(NC):
        sel = np.nonzero(core_of == k)[0]
        ef, et = nf[sel], nt[sel] - k * SLICE
        deg = np.bincount(et, minlength=SLICE)
        order = np.argsort(-deg, kind="stable")       # rank -> node(local)
        rank_of = np.empty(SLICE, np.int64)
        rank_of[order] = np.arange(SLICE)
        r_e = rank_of[et]
        # j index within each destination: stable sort by rank, then enumerate
        s = np.argsort(r_e, kind="stable")
        rs = r_e[s]
        first = np.r_[True, rs[1:] != rs[:-1]]
        grp_start = np.maximum.accumulate(np.where(first, np.arange(len(rs)), 0))
        j_e = np.arange(len(rs)) - grp_start
        # layer sizes
        dmax = int(deg.max()) if len(deg) else 0
        hist = np.bincount(deg, minlength=dmax + 2)
        cj = SLICE - np.cumsum(hist)[:-1]             # cj[j] = #{deg > j}
        cj = cj[:dmax]
        all_cj.append(cj)
        per_core.append(dict(sel=sel[s], r=rs, j=j_e, order=order,
                             rank_of=rank_of, deg=deg))
    dmax_g = max(len(c) for c in all_cj)
    CJ = np.zeros(dmax_g, np.int64)
    for c in all_cj:
        CJ[:len(c)] = np.maximum(CJ[:len(c)], c)
    ncols = (CJ + 127) // 128
    bases = np.r_[0, np.cumsum(ncols)]
    TC = int(bases[-1])
    layers = [(int(bases[j]), int(ncols[j])) for j in range(dmax_g)]

    # per-core slot arrays
    cores = []
    for k in range(NC):
        pc = per_core[k]
        p = pc["r"] % 128
        col = bases[pc["j"]] + pc["r"] // 128
        e = pc["sel"]
        from_g = np.full((128, TC), -1, np.int64)
        from_g[p, col] = nf[e]
        ln = np.zeros((128, TC), np.float32)
        ln[p, col] = elen[e]
        cf = np.zeros((128, TC, 3), np.float32)
        cf[p, col] = coords[nf[e]]
        ct = np.zeros((128, TC, 3), np.float32)
        ct[p, col] = coords[nt[e]]
        ev = np.zeros((128, TC, 3), np.float32)
        ev[p, col] = evec[e]
        # gather index into concat-of-slabs [NC*SLAB+1] (rank space), pads->zero row
        own = nf[e] // SLICE
        rk = np.concatenate([per_core[c]["rank_of"] for c in range(NC)]
                            ).reshape(NC, SLICE)
        gidx = np.full((128, TC), NC * SLAB, np.int64)
        gidx[p, col] = own * SLAB + rk[own, nf[e] % SLICE]
        real = from_g >= 0
        cores.append(dict(ln=ln, cf=cf, ct=ct, ev=ev, gidx=gidx, real=real,
                          rank_of=pc["rank_of"], order=pc["order"]))

    # graph stage structure (single core)
    m_g = np.bincount(ngi, minlength=G)
    gorder = np.argsort(-m_g, kind="stable")          # grank -> graph
    grank_of = np.empty(G, np.int64)
    grank_of[gorder] = np.arange(G)
    gr_n = grank_of[ngi]                              # per node
    s = np.argsort(gr_n, kind="stable")
    grs = gr_n[s]
    first = np.r_[True, grs[1:] != grs[:-1]]
    grp_start = np.maximum.accumulate(np.where(first, np.arange(len(grs)), 0))
    gi_n = np.arange(len(grs)) - grp_start
    gmax = int(m_g.max())
    ghist = np.bincount(m_g, minlength=gmax + 2)
    gcj = G - np.cumsum(ghist)[:-1]
    gcj = gcj[:gmax]
    gncols = (gcj + 127) // 128
    gbases = np.r_[0, np.cumsum(gncols)]
    TG = int(gbases[-1])
    glayers = [(int(gbases[i]), int(gncols[i])) for i in range(gmax)]
    # node (global, in s order) -> slot (p, col); row id into state concat
    nodes_s = s
    own = nodes_s // SLICE
    loc = nodes_s % SLICE
    rk = np.stack([per_core[c]["rank_of"] for c in range(NC)])
    rowid = np.full((128, TG), NC * SLAB, np.int64)
    gp_ = grs % 128
    gcol = gbases[gi_n] + grs // 128
    rowid[gp_, gcol] = own * SLAB + rk[own, loc]
    return dict(TC=TC, layers=layers, cores=cores,
                TG=TG, glayers=glayers, rowid=rowid, gorder=gorder)


# ---------------------------------------------------------------- kernel

def kernel(node_coordinates, edge_lengths, edge_vectors, W_msg, b_msg,
           W_out, b_out, node_from, node_to, node_graph_index,
           num_nodes, num_graphs):
    global LAST_EXEC_NS
    LAST_EXEC_NS = 0
    _maybe_install_trace_hook()
    assert int(num_nodes) == N and int(num_graphs) == G

    W_msg = np.asarray(W_msg, np.float32)
    b_msg = np.asarray(b_msg, np.float32)
    W_out = np.asarray(W_out, np.float32)
    b_out = np.asarray(b_out, np.float32)

    pp = _prep(node_coordinates, edge_lengths, edge_vectors,
               node_from, node_to, node_graph_index)
    TC, layers, cores = pp["TC"], pp["layers"], pp["cores"]

    # --- geo phase
    wg = np.tile(W_msg[10:14].reshape(1, 40), (128, 1)).astype(np.float32)
    nc_geo = build_geo(TC)
    maps = [{"len": c["ln"], "cf": c["cf"], "ct": c["ct"], "ev": c["ev"],
             "wg": wg} for c in cores]
    res = _run(nc_geo, maps, list(range(NC)))
    gps = [r["gp"] for r in res]

    # --- rounds
    ws = np.tile(np.concatenate([W_msg[:10].reshape(-1), b_msg]).reshape(1, 110),
                 (128, 1)).astype(np.float32)
    nc_round = build_round(TC, layers)
    state = [np.zeros((128, SLABC, 10), np.float32) for _ in range(NC)]
    # round 1: gath = b_msg at real slots (z = gp + b)
    gath = []
    for c in cores:
        g = np.zeros((128, TC, 10), np.float32)
        g[c["real"]] = b_msg
        gath.append(g)
    for r in range(3):
        maps = [{"gp": gps[k], "gath": gath[k], "state_in": state[k],
                 "ws": ws} for k in range(NC)]
        res = _run(nc_round, maps, list(range(NC)))
        state = [x["state"] for x in res]
        if r < 2:
            proj = np.concatenate(
                [x["proj"].transpose(1, 0, 2).reshape(SLAB, 10) for x in res]
                + [np.zeros((1, 10), np.float32)])
            # slab row layout: rank r at (r%128, r//128) -> flat r = col*128+p
            # transpose(1,0,2) gives [col, p] order = rank-major? No:
            # rank = p + 128*col -> need [p + 128*col] ordering
            gath = [proj[c["gidx"]] for c in cores]

    # --- graph stage (single core)
    statecat = np.concatenate(
        [s.transpose(1, 0, 2).reshape(SLAB, 10) for s in state]
        + [np.zeros((1, 10), np.float32)])
    rows = statecat[pp["rowid"]]
    wo = np.tile(np.concatenate([W_out.reshape(-1), b_out]).reshape(1, 44),
                 (128, 1)).astype(np.float32)
    nc_fin = build_final(pp["TG"], pp["glayers"])
    res = _run(nc_fin, [{"rows": rows.astype(np.float32), "wo": wo}], [0])
    evout = res[0]["evout"]
    # graph grank at (grank%128, grank//128); map back to graph id
    out = np.zeros((G, 4), np.float32)
    gr = np.arange(G)
    out[pp["gorder"]] = evout[gr % 128, gr // 128, :]
    return out
